# revision 21
# baseline (speedup 1.0000x reference)
"""Distributed Bass kernel: fused multi-head attention block on 8 TRN2 NeuronCores.

Problem: x[2,2048,1024] -> QKV proj -> RoPE(q,k) -> softmax(q k^T/8) v -> out proj.

Sharding: tensor-parallel over heads. 16 heads / 8 cores = 2 heads per core.
Each core computes QKV for its 2 heads (full sequence), RoPE, attention, then
an AllToAll converts head-sharding to token-sharding so the output projection
runs against the FULL Wout with no AllReduce. The AllToAll is split in two
(one per batch) so the first collective is hidden under batch-1 attention and
the second under the half-0 output projection.

Pipeline (single pass, emission order == engine order):
  1. QKV+RoPE for batch-0 chunks (tokens 0..2047), DMA-prefetched.
  2. batch-0 attention chunks interleaved with batch-1 QKV chunks: the
     scores/exp of query-chunk i zippered with the PV matmuls of chunk i-1
     and with slivers of QKV work so PE, ACT and DVE all stay fed.
  3. A2A#0 launches as soon as batch-0's attention output is normalized;
     batch-1 attention runs during the collective.
  4. Tail: A2A#1 runs concurrently with the half-0 output projection.

PE geometry notes:
  - scores contract over d=64 per head; the two heads run as CONCURRENT
    row-tiled matmuls (tile_position rows 0/64) -> ~2x score throughput.
  - v is produced token-major directly from the QKV matmul (lhsT = x tile),
    so no PE transposes are needed; per-head v tables carry a ones column
    so the PV matmul also accumulates the softmax denominator.

Compute dtype bf16 (PE 1 cycle/row), f32 PSUM accumulation. Softmax skips the
max-subtraction (scores ~N(0,2), |s|<~12, exp safe in f32) and folds the
denominator into the PV matmul via the ones column.
"""

import sys

for _p in ("/opt/trn_rl_repo", "/root/.axon_site/_ro/trn_rl_repo"):
    if _p not in sys.path:
        sys.path.append(_p)

import numpy as np
import ml_dtypes

B, N, HID = 2, 2048, 1024
H, DH = 16, 64
NCORES = 8
HPC = H // NCORES          # heads per core = 2
T = B * N                  # 4096 flattened tokens
EPC = HPC * DH             # 128 features per core
CH = 512                   # token chunk for QKV phase
NCH = T // CH              # 8 chunks
KT = 128                   # key tile
QC = 512                   # query chunk in attention
NKT = N // KT              # 16 key tiles per batch
NPAIR = NKT // 2           # 8 kt-pairs per query chunk
THALF = N                  # tokens per collective half (one batch)
TSH = THALF // NCORES      # 256 tokens per core per half
TS = 2 * TSH               # 512 output tokens per core

_bf16 = ml_dtypes.bfloat16


def _build_graph():
    import concourse.bass as bass
    import concourse.mybir as mybir
    import concourse.tile as tile
    from concourse import bacc

    f32 = mybir.dt.float32
    bf16 = mybir.dt.bfloat16

    nc = bacc.Bacc("TRN2", target_bir_lowering=False, debug=False, num_devices=NCORES)

    xT_e = nc.declare_dram_parameter("xT", [HID, T], bf16, isOutput=False)
    wqkvT_e = nc.declare_dram_parameter("wqkvT", [HID, 3 * EPC], bf16, isOutput=False)
    woutT_e = nc.declare_dram_parameter("woutT", [HID, HID], bf16, isOutput=False)
    cos2_e = nc.declare_dram_parameter("cos2", [2 * DH, T], bf16, isOutput=False)
    sin2_e = nc.declare_dram_parameter("sin2", [2 * DH, T], bf16, isOutput=False)
    perm_e = nc.declare_dram_parameter("perm", [128, 128], bf16, isOutput=False)
    out_e = nc.declare_dram_parameter("out", [TS, HID], f32, isOutput=True)

    with tile.TileContext(nc) as tc:
        with (
            tc.tile_pool(name="const", bufs=1) as cpool,
            tc.tile_pool(name="work", bufs=1) as wpool,
            tc.tile_pool(name="stream", bufs=4) as spool,
            tc.tile_pool(name="psum", bufs=2, space="PSUM") as pspool,
            tc.tile_pool(name="dram", bufs=1, space="DRAM") as dpool,
        ):
            # ---- constants / weights (single batched DMAs) ----
            wqkvT = cpool.tile([128, 8 * 3 * EPC], bf16)       # 8 k-tiles side by side
            nc.gpsimd.dma_start(
                wqkvT.rearrange("p (kt c) -> p kt c", c=3 * EPC),
                wqkvT_e.rearrange("(kt p) c -> p kt c", p=128),
            )
            perm = cpool.tile([128, 128], bf16)
            nc.gpsimd.dma_start(perm[:, :], perm_e[:, :])
            woutT = cpool.tile([128, 8 * HID], bf16)
            cos2 = cpool.tile([128, T], bf16)
            sin2 = cpool.tile([128, T], bf16)

            # ---- persistent working tensors ----
            k_sb = wpool.tile([128, T], bf16)      # roped k, feature-major
            q_roped = wpool.tile([128, T], bf16)   # roped q, feature-major
            vexC = wpool.tile([128, 32 * 128], bf16)  # per-slot [vA(64)|vB(64)]
            ovT = wpool.tile([128, T], bf16)       # attention out ^T
            onesT = wpool.tile([128, 32], bf16)    # lhsT for denominator sums
            junk = wpool.tile([1, 8], f32)

            vexC3 = vexC.rearrange("p (s c) -> p s c", c=128)
            nc.vector.memset(onesT[:, :], 1.0)
            # preload the exp ACT table set while startup DMAs run
            nc.vector.memset(junk[:, :], 0.0)
            nc.scalar.activation(
                junk[:, :], junk[:, :], mybir.ActivationFunctionType.Exp
            )

            xT_e3 = xT_e.rearrange("(kt p) t -> p kt t", p=128)

            # ---------------- QKV chunk (as a list of filler closures) --------
            def qkv_fillers(c):
                """Return closures emitting QKV+RoPE for chunk c in ~6 slices."""
                sl = slice(c * CH, (c + 1) * CH)
                st = {}

                def load_x():
                    xc = spool.tile([128, 8 * CH], bf16, tag="xc", bufs=3)
                    nc.gpsimd.dma_start(
                        xc.rearrange("p (kt t) -> p kt t", t=CH),
                        xT_e3[:, :, sl],
                    )
                    st["xc"] = xc

                def proj(which, dest_raw):
                    # feature-major q or k: [128 = dA|dB, CH tokens]
                    xc = st["xc"]
                    ps = pspool.tile([128, CH], f32, tag="mm", bufs=2)
                    for kt in range(8):
                        nc.tensor.matmul(
                            ps[:, :],
                            wqkvT[:, kt * 3 * EPC + which * EPC:
                                  kt * 3 * EPC + (which + 1) * EPC],
                            xc[:, kt * CH:(kt + 1) * CH],
                            start=(kt == 0),
                            stop=(kt == 7),
                        )
                    nc.vector.tensor_copy(dest_raw, ps[:, :])

                def rope(src, dest):
                    # dest = src*cos + (P@src)*sin   (sin carries the sign flip)
                    pps = pspool.tile([128, CH], f32, tag="mm", bufs=2)
                    nc.tensor.matmul(pps[:, :], perm[:, :], src, start=True, stop=True)
                    tmp = spool.tile([128, CH], bf16, tag="rtmp", bufs=2)
                    nc.vector.tensor_mul(tmp[:, :], pps[:, :], sin2[:, sl])
                    nc.vector.tensor_mul(dest, src, cos2[:, sl])
                    nc.vector.tensor_add(dest, dest, tmp[:, :])

                def q_step():
                    qr = spool.tile([128, CH], bf16, tag="qraw", bufs=2)
                    st["qr"] = qr
                    proj(0, qr[:, :])
                    rope(qr[:, :], q_roped[:, sl])

                def k_step():
                    proj(1, k_sb[:, sl])
                    rope(k_sb[:, sl], k_sb[:, sl])

                def v_step(tt):
                    # token-major v for 128 tokens, both heads in one matmul
                    xc = st["xc"]
                    slot = c * (CH // 128) + tt
                    vps = pspool.tile([128, 128], f32, tag="mm", bufs=2)
                    for kt in range(8):
                        nc.tensor.matmul(
                            vps[:, :],
                            xc[:, kt * CH + tt * 128:kt * CH + tt * 128 + 128],
                            wqkvT[:, kt * 3 * EPC + 2 * EPC:
                                  kt * 3 * EPC + 3 * EPC],
                            start=(kt == 0),
                            stop=(kt == 7),
                        )
                    nc.vector.tensor_copy(vexC3[:, slot, :], vps[:, :])

                return [
                    load_x,
                    q_step,
                    k_step,
                    lambda: v_step(0),
                    lambda: v_step(1),
                    lambda: v_step(2),
                    lambda: v_step(3),
                ]

            # ---------------- attention helpers ------------------------------
            def emit_pv_kt(st, kt):
                # Both heads' PV matmuls run as concurrent COL-tiled matmuls
                # (M=64 each, output partitions 0-63 / 64-127 of one bank).
                (b, qc, opsC, psD, expT) = st
                slot = b * NKT + kt
                for h in range(HPC):
                    nc.tensor.matmul(
                        opsC[h * DH:(h + 1) * DH, :],
                        vexC3[:, slot, h * DH:(h + 1) * DH],
                        expT[:, (2 * kt + h) * QC:(2 * kt + h + 1) * QC],
                        start=(kt == 0),
                        stop=(kt == NKT - 1),
                    )
                if kt % 2 == 1:
                    # softmax denominators for the pair (kt-1, kt): four
                    # concurrent M=32 col-tiled all-ones matmuls. Rows
                    # 0:32 / 32:64 accumulate head A/B over even kts,
                    # rows 64:96 / 96:128 over odd kts.
                    pair = kt // 2
                    for j, (kk, h) in enumerate(
                            ((kt - 1, 0), (kt - 1, 1), (kt, 0), (kt, 1))):
                        nc.tensor.matmul(
                            psD[j * 32:(j + 1) * 32, :],
                            onesT[:, :],
                            expT[:, (2 * kk + h) * QC:(2 * kk + h + 1) * QC],
                            start=(pair == 0),
                            stop=(pair == NKT // 2 - 1),
                            tile_position=(0, 32 * j),
                        )

            def emit_normalize(st):
                (b, qc, opsC, psD, expT) = st
                q0 = b * N + qc * QC
                for h in range(HPC):
                    hr = h * DH
                    den = spool.tile([1, QC], f32, tag="den", bufs=2)
                    nc.vector.tensor_copy(den[0:1, :], psD[32 * h:32 * h + 1, :])
                    nc.vector.tensor_add(
                        den[0:1, :], den[0:1, :], psD[64 + 32 * h:64 + 32 * h + 1, :]
                    )
                    rec = spool.tile([1, QC], f32, tag="rec", bufs=2)
                    nc.vector.reciprocal_approx_fast(rec[0:1, :], den[0:1, :])
                    bcs = spool.tile([64, QC], f32, tag="bcs", bufs=2)
                    nc.gpsimd.partition_broadcast(bcs[:, :], rec[0:1, :])
                    nc.vector.tensor_mul(
                        ovT[hr:hr + DH, q0:q0 + QC], opsC[hr:hr + DH, :], bcs[:, :]
                    )

            def emit_attn_chunk(b, qc, pending, fillers=()):
                """Scores+exp for (b, qc), zippered with PV of `pending` and
                with QKV filler slices. Returns the new pending state."""
                q0 = b * N + qc * QC
                fillers = list(fillers)
                expT = spool.tile([128, NKT * 2 * QC], bf16, name="expT",
                                  tag="expT", bufs=2)
                for kt in range(NKT):
                    # both heads' score matmuls share one PSUM tile and one
                    # exp so they become ready together and stay adjacent:
                    # they are row-tiled (rows 0-63 / 64-127) and stream the
                    # same q columns, so the PE can run them concurrently.
                    psS = pspool.tile([128, 2 * QC], f32, name="psS",
                                      tag="sc", bufs=2)
                    k0 = b * N + kt * KT
                    for h in range(HPC):
                        p0, p1 = h * DH, (h + 1) * DH
                        nc.tensor.matmul(
                            psS[:, h * QC:(h + 1) * QC],
                            k_sb[p0:p1, k0:k0 + KT],
                            q_roped[p0:p1, q0:q0 + QC],
                            start=True, stop=True,
                        )
                    if fillers and kt % 2 == 0:
                        fillers.pop(0)()
                    nc.scalar.activation(
                        expT[:, 2 * kt * QC:(2 * kt + 2) * QC],
                        psS[:, :],
                        mybir.ActivationFunctionType.Exp,
                        scale=DH ** -0.5,
                    )
                    if pending is not None:
                        emit_pv_kt(pending, kt)
                        if kt == NKT - 1:
                            emit_normalize(pending)
                while fillers:
                    fillers.pop(0)()
                opsC = pspool.tile([128, QC], f32, name="opsC",
                                   tag="ops", bufs=2)
                psD = pspool.tile([128, QC], f32, name="psD",
                                  tag="ops", bufs=2)
                return (b, qc, opsC, psD, expT)

            # ---------------- collective / output helpers ---------------------
            # The A2A is split into 4 quarter-collectives (1024 tokens each),
            # launched as soon as each quarter of attention output is
            # normalized, so all but the last hide under remaining compute.
            TQ = T // 4                  # 1024 tokens per quarter
            TSQ = TQ // NCORES           # 128 tokens per core per quarter
            a2a_in = [dpool.tile([NCORES * 128, TSQ], bf16, name=f"a2a_in{q}")
                      for q in range(4)]
            a2a_out = [dpool.tile([NCORES * 128, TSQ], bf16, name=f"a2a_out{q}")
                       for q in range(4)]
            gTs = [wpool.tile([128, 8 * TSQ], bf16, name=f"gTq{q}")
                   for q in range(4)]

            def emit_a2a(q):
                nc.gpsimd.dma_start(
                    a2a_in[q].rearrange("(j p) t -> p j t", p=128),
                    ovT[:, q * TQ:(q + 1) * TQ]
                    .rearrange("p (j t) -> p j t", t=TSQ),
                )
                nc.gpsimd.collective_compute(
                    "AllToAll",
                    mybir.AluOpType.bypass,
                    ins=[a2a_in[q].opt()],
                    outs=[a2a_out[q].opt()],
                    replica_groups=[list(range(NCORES))],
                )

            def emit_gather(q, fence_tok=None):
                # fence_tok: an ovT token index whose normalize must precede
                # the gather. Keeps the scheduler from hoisting the gather
                # (and the out-proj matmuls behind it) to a point where a
                # wait on the collective would stall the in-order PE queue.
                if fence_tok is not None:
                    nc.vector.tensor_copy(
                        gTs[q][0:1, 0:8], ovT[0:1, fence_tok - 8:fence_tok]
                    )
                nc.gpsimd.dma_start(
                    gTs[q].rearrange("p (e t) -> p e t", t=TSQ),
                    a2a_out[q].rearrange("(e p) t -> p e t", p=128),
                )

            def emit_outproj(q):
                gT = gTs[q]
                for nn in range(HID // 512):
                    odps = pspool.tile([128, 512], f32, tag="mm", bufs=2)
                    for et in range(8):
                        nc.tensor.matmul(
                            odps[:, :],
                            gT[:, et * TSQ:(et + 1) * TSQ],
                            woutT[:, et * HID + nn * 512:
                                  et * HID + (nn + 1) * 512],
                            start=(et == 0),
                            stop=(et == 7),
                        )
                    osb = spool.tile([128, 512], f32, tag="osb", bufs=2)
                    nc.vector.tensor_copy(osb[:, :], odps[:, :])
                    nc.gpsimd.dma_start(
                        out_e[q * TSQ:(q + 1) * TSQ, nn * 512:(nn + 1) * 512],
                        osb[:, :],
                    )

            # ======================= emission schedule =======================
            # batch-0 QKV (chunks 0-3); chunk 0's x DMA goes ahead of the
            # (larger) rope tables so the first matmul starts ASAP.
            f0 = qkv_fillers(0)
            f0.pop(0)()                  # load_x(0)
            nc.gpsimd.dma_start(cos2[:, :], cos2_e[:, :])
            nc.gpsimd.dma_start(sin2[:, :], sin2_e[:, :])
            for f in f0:
                f()
            for c in range(1, 4):
                for f in qkv_fillers(c):
                    f()
                if c == 1:
                    nc.gpsimd.dma_start(
                        woutT.rearrange("p (kt c) -> p kt c", c=HID),
                        woutT_e.rearrange("(kt p) c -> p kt c", p=128),
                    )

            # batch-0 attention interleaved with batch-1 QKV (chunks 4-7)
            pending = None
            for qc in range(N // QC):
                pending = emit_attn_chunk(0, qc, pending,
                                          fillers=qkv_fillers(4 + qc))
                if qc == 2:
                    emit_a2a(0)          # tokens 0:1024 normalized during qc 2

            # batch-1 attention; each remaining quarter's A2A launches as its
            # tokens are normalized; each out-proj quarter is fenced one
            # normalize behind so it fills PE slack in the ACT-bound region.
            for qc in range(N // QC):
                pending = emit_attn_chunk(1, qc, pending)
                if qc == 0:
                    emit_a2a(1)          # tokens 1024:2048 (norm'd during qc 0)
                if qc == 1:
                    emit_gather(0, fence_tok=N + QC)
                    emit_outproj(0)
                if qc == 2:
                    emit_a2a(2)          # tokens 2048:3072 (norm'd during qc 2)
                    emit_gather(1, fence_tok=N + 2 * QC)
                    emit_outproj(1)
            for kt in range(NKT):
                emit_pv_kt(pending, kt)
            emit_normalize(pending)
            emit_a2a(3)                  # tokens 3072:4096
            emit_gather(2, fence_tok=N + 3 * QC)
            emit_outproj(2)              # runs while A2A#3 is in flight
            emit_gather(3, fence_tok=T)
            emit_outproj(3)

    nc.finalize()
    return nc


def _host_inputs(x, rope, Wqkv, Wout):
    """Build the 8 per-core input maps with host-side layout prep."""
    xf = np.ascontiguousarray(x.reshape(T, HID).T).astype(_bf16)        # [1024, 4096]
    woutT = np.ascontiguousarray(Wout.T).astype(_bf16)                  # [1024, 1024]

    rf = rope.reshape(T, DH)                                            # [4096, 64]
    cosE = np.repeat(rf[:, 0::2], 2, axis=1).T                          # [64, 4096]
    sinE = np.repeat(rf[:, 1::2], 2, axis=1).T
    sgn = np.where(np.arange(DH) % 2 == 0, -1.0, 1.0)[:, None]
    sinS = (sinE * sgn)
    cos2 = np.ascontiguousarray(np.concatenate([cosE, cosE], 0)).astype(_bf16)
    sin2 = np.ascontiguousarray(np.concatenate([sinS, sinS], 0)).astype(_bf16)

    pm = np.zeros((128, 128), np.float32)
    for d in range(128):
        pm[d ^ 1, d] = 1.0       # partner[d] = q[d^1]; lhsT = S (symmetric)
    perm = pm.astype(_bf16)

    w3 = Wqkv.reshape(3, H, DH, HID)
    in_maps = []
    for c in range(NCORES):
        blocks = []
        for which in range(3):
            for hl in range(HPC):
                blocks.append(w3[which, 2 * c + hl])                    # [64, 1024]
        wq = np.concatenate(blocks, 0)                                  # [384, 1024]
        wqkvT = np.ascontiguousarray(wq.T).astype(_bf16)                # [1024, 384]
        in_maps.append({
            "xT": xf, "wqkvT": wqkvT, "woutT": woutT,
            "cos2": cos2, "sin2": sin2, "perm": perm,
        })
    return in_maps


_CACHE = {}


def kernel(x, rope, Wqkv, Wout):
    from concourse.bass_utils import run_bass_kernel_spmd

    if "nc" not in _CACHE:
        _CACHE["nc"] = _build_graph()
    nc = _CACHE["nc"]
    in_maps = _host_inputs(np.asarray(x, np.float32), np.asarray(rope, np.float32),
                           np.asarray(Wqkv, np.float32), np.asarray(Wout, np.float32))
    res = run_bass_kernel_spmd(nc, in_maps, core_ids=list(range(NCORES)))
    # core j returns [512, 1024]: rows q*128:(q+1)*128 hold global tokens
    # [q*1024 + j*128, q*1024 + (j+1)*128) for quarter q.
    full = np.empty((T, HID), np.float32)
    TQ, TSQ = T // 4, T // 4 // NCORES
    for j in range(NCORES):
        part = np.asarray(res.results[j]["out"], np.float32)
        for q in range(4):
            full[q * TQ + j * TSQ:q * TQ + (j + 1) * TSQ] = \
                part[q * TSQ:(q + 1) * TSQ]
    return full.reshape(B, N, HID)


# revision 22
# speedup vs baseline: 1.1713x; 1.1713x over previous
"""Distributed Bass kernel: fused multi-head attention block on 8 TRN2 NeuronCores.

Problem: x[2,2048,1024] -> QKV proj -> RoPE(q,k) -> softmax(q k^T/8) v -> out proj.

Sharding: tensor-parallel over heads. 16 heads / 8 cores = 2 heads per core.
Each core computes QKV for its 2 heads (full sequence), RoPE, attention, then
an AllToAll converts head-sharding to token-sharding so the output projection
runs against the FULL Wout with no AllReduce. The AllToAll is split in two
(one per batch) so the first collective is hidden under batch-1 attention and
the second under the half-0 output projection.

Pipeline (single pass, emission order == engine order):
  1. QKV+RoPE for batch-0 chunks (tokens 0..2047), DMA-prefetched.
  2. batch-0 attention chunks interleaved with batch-1 QKV chunks: the
     scores/exp of query-chunk i zippered with the PV matmuls of chunk i-1
     and with slivers of QKV work so PE, ACT and DVE all stay fed.
  3. A2A#0 launches as soon as batch-0's attention output is normalized;
     batch-1 attention runs during the collective.
  4. Tail: A2A#1 runs concurrently with the half-0 output projection.

PE geometry notes:
  - scores contract over d=64 per head; the two heads run as CONCURRENT
    row-tiled matmuls (tile_position rows 0/64) -> ~2x score throughput.
  - v is produced token-major directly from the QKV matmul (lhsT = x tile),
    so no PE transposes are needed; per-head v tables carry a ones column
    so the PV matmul also accumulates the softmax denominator.

Compute dtype bf16 (PE 1 cycle/row), f32 PSUM accumulation. Softmax skips the
max-subtraction (scores ~N(0,2), |s|<~12, exp safe in f32) and folds the
denominator into the PV matmul via the ones column.
"""

import sys

for _p in ("/opt/trn_rl_repo", "/root/.axon_site/_ro/trn_rl_repo"):
    if _p not in sys.path:
        sys.path.append(_p)

import numpy as np
import ml_dtypes

B, N, HID = 2, 2048, 1024
H, DH = 16, 64
NCORES = 8
HPC = H // NCORES          # heads per core = 2
T = B * N                  # 4096 flattened tokens
EPC = HPC * DH             # 128 features per core
CH = 512                   # token chunk for QKV phase
NCH = T // CH              # 8 chunks
KT = 128                   # key tile
QC = 512                   # query chunk in attention
NKT = N // KT              # 16 key tiles per batch
NPAIR = NKT // 2           # 8 kt-pairs per query chunk
THALF = N                  # tokens per collective half (one batch)
TSH = THALF // NCORES      # 256 tokens per core per half
TS = 2 * TSH               # 512 output tokens per core

_bf16 = ml_dtypes.bfloat16


def _build_graph():
    import concourse.bass as bass
    import concourse.mybir as mybir
    import concourse.tile as tile
    from concourse import bacc

    f32 = mybir.dt.float32
    bf16 = mybir.dt.bfloat16

    nc = bacc.Bacc("TRN2", target_bir_lowering=False, debug=False, num_devices=NCORES)

    xT_e = nc.declare_dram_parameter("xT", [HID, T], bf16, isOutput=False)
    wqkvT_e = nc.declare_dram_parameter("wqkvT", [HID, 3 * EPC], bf16, isOutput=False)
    woutT_e = nc.declare_dram_parameter("woutT", [HID, HID], bf16, isOutput=False)
    cos2_e = nc.declare_dram_parameter("cos2", [2 * DH, T], bf16, isOutput=False)
    sin2_e = nc.declare_dram_parameter("sin2", [2 * DH, T], bf16, isOutput=False)
    perm_e = nc.declare_dram_parameter("perm", [128, 128], bf16, isOutput=False)
    out_e = nc.declare_dram_parameter("out", [TS, HID], f32, isOutput=True)

    with tile.TileContext(nc) as tc:
        with (
            tc.tile_pool(name="const", bufs=1) as cpool,
            tc.tile_pool(name="work", bufs=1) as wpool,
            tc.tile_pool(name="stream", bufs=4) as spool,
            tc.tile_pool(name="psum", bufs=2, space="PSUM") as pspool,
            tc.tile_pool(name="dram", bufs=1, space="DRAM") as dpool,
        ):
            # ---- constants / weights (single batched DMAs) ----
            wqkvT = cpool.tile([128, 8 * 3 * EPC], bf16)       # 8 k-tiles side by side
            nc.gpsimd.dma_start(
                wqkvT.rearrange("p (kt c) -> p kt c", c=3 * EPC),
                wqkvT_e.rearrange("(kt p) c -> p kt c", p=128),
            )
            perm = cpool.tile([128, 128], bf16)
            nc.gpsimd.dma_start(perm[:, :], perm_e[:, :])
            woutT = cpool.tile([128, 8 * HID], bf16)
            cos2 = cpool.tile([128, T], bf16)
            sin2 = cpool.tile([128, T], bf16)

            # ---- persistent working tensors ----
            k_sb = wpool.tile([128, T], bf16)      # roped k, feature-major
            q_roped = wpool.tile([128, T], bf16)   # roped q, feature-major
            vexC = wpool.tile([128, 32 * 128], bf16)  # per-slot [vA(64)|vB(64)]
            ovT = wpool.tile([128, T], bf16)       # attention out ^T
            onesT = wpool.tile([128, 32], bf16)    # lhsT for denominator sums
            junk = wpool.tile([1, 8], f32)

            vexC3 = vexC.rearrange("p (s c) -> p s c", c=128)
            nc.vector.memset(onesT[:, :], 1.0)
            # preload the exp ACT table set while startup DMAs run
            nc.vector.memset(junk[:, :], 0.0)
            nc.scalar.activation(
                junk[:, :], junk[:, :], mybir.ActivationFunctionType.Exp
            )

            xT_e3 = xT_e.rearrange("(kt p) t -> p kt t", p=128)

            # ---------------- QKV chunk (as a list of filler closures) --------
            def qkv_fillers(c):
                """Return closures emitting QKV+RoPE for chunk c in ~6 slices."""
                sl = slice(c * CH, (c + 1) * CH)
                st = {}

                def load_x():
                    xc = spool.tile([128, 8 * CH], bf16, tag="xc", bufs=3)
                    nc.gpsimd.dma_start(
                        xc.rearrange("p (kt t) -> p kt t", t=CH),
                        xT_e3[:, :, sl],
                    )
                    st["xc"] = xc

                def proj(which, dest_raw):
                    # feature-major q or k: [128 = dA|dB, CH tokens]
                    xc = st["xc"]
                    ps = pspool.tile([128, CH], f32, tag="mm", bufs=2)
                    for kt in range(8):
                        nc.tensor.matmul(
                            ps[:, :],
                            wqkvT[:, kt * 3 * EPC + which * EPC:
                                  kt * 3 * EPC + (which + 1) * EPC],
                            xc[:, kt * CH:(kt + 1) * CH],
                            start=(kt == 0),
                            stop=(kt == 7),
                        )
                    nc.vector.tensor_copy(dest_raw, ps[:, :])

                def rope(src, dest):
                    # dest = src*cos + (P@src)*sin   (sin carries the sign flip)
                    pps = pspool.tile([128, CH], f32, tag="mm", bufs=2)
                    nc.tensor.matmul(pps[:, :], perm[:, :], src, start=True, stop=True)
                    tmp = spool.tile([128, CH], bf16, tag="rtmp", bufs=2)
                    nc.vector.tensor_mul(tmp[:, :], pps[:, :], sin2[:, sl])
                    nc.vector.tensor_mul(dest, src, cos2[:, sl])
                    nc.vector.tensor_add(dest, dest, tmp[:, :])

                def q_step():
                    qr = spool.tile([128, CH], bf16, tag="qraw", bufs=2)
                    st["qr"] = qr
                    proj(0, qr[:, :])
                    rope(qr[:, :], q_roped[:, sl])

                def k_step():
                    proj(1, k_sb[:, sl])
                    rope(k_sb[:, sl], k_sb[:, sl])

                def v_step(tt):
                    # token-major v for 128 tokens, both heads in one matmul
                    xc = st["xc"]
                    slot = c * (CH // 128) + tt
                    vps = pspool.tile([128, 128], f32, tag="mm", bufs=2)
                    for kt in range(8):
                        nc.tensor.matmul(
                            vps[:, :],
                            xc[:, kt * CH + tt * 128:kt * CH + tt * 128 + 128],
                            wqkvT[:, kt * 3 * EPC + 2 * EPC:
                                  kt * 3 * EPC + 3 * EPC],
                            start=(kt == 0),
                            stop=(kt == 7),
                        )
                    nc.vector.tensor_copy(vexC3[:, slot, :], vps[:, :])

                return [
                    load_x,
                    q_step,
                    k_step,
                    lambda: v_step(0),
                    lambda: v_step(1),
                    lambda: v_step(2),
                    lambda: v_step(3),
                ]

            # ---------------- attention helpers ------------------------------
            def emit_pv_kt(st, kt):
                # Both heads' PV matmuls run as concurrent COL-tiled matmuls
                # (M=64 each, output partitions 0-63 / 64-127 of one bank).
                (b, qc, opsC, psD, expT) = st
                slot = b * NKT + kt
                for h in range(HPC):
                    nc.tensor.matmul(
                        opsC[h * DH:(h + 1) * DH, :],
                        vexC3[:, slot, h * DH:(h + 1) * DH],
                        expT[:, (2 * kt + h) * QC:(2 * kt + h + 1) * QC],
                        start=(kt == 0),
                        stop=(kt == NKT - 1),
                    )
                if kt % 2 == 1:
                    # softmax denominators for the pair (kt-1, kt): four
                    # concurrent M=32 col-tiled all-ones matmuls. Rows
                    # 0:32 / 32:64 accumulate head A/B over even kts,
                    # rows 64:96 / 96:128 over odd kts.
                    pair = kt // 2
                    for j, (kk, h) in enumerate(
                            ((kt - 1, 0), (kt - 1, 1), (kt, 0), (kt, 1))):
                        nc.tensor.matmul(
                            psD[j * 32:(j + 1) * 32, :],
                            onesT[:, :],
                            expT[:, (2 * kk + h) * QC:(2 * kk + h + 1) * QC],
                            start=(pair == 0),
                            stop=(pair == NKT // 2 - 1),
                            tile_position=(0, 32 * j),
                        )

            def emit_normalize(st):
                (b, qc, opsC, psD, expT) = st
                q0 = b * N + qc * QC
                for h in range(HPC):
                    hr = h * DH
                    den = spool.tile([1, QC], f32, tag="den", bufs=2)
                    nc.vector.tensor_copy(den[0:1, :], psD[32 * h:32 * h + 1, :])
                    nc.vector.tensor_add(
                        den[0:1, :], den[0:1, :], psD[64 + 32 * h:64 + 32 * h + 1, :]
                    )
                    rec = spool.tile([1, QC], f32, tag="rec", bufs=2)
                    nc.vector.reciprocal_approx_fast(rec[0:1, :], den[0:1, :])
                    bcs = spool.tile([64, QC], f32, tag="bcs", bufs=2)
                    nc.gpsimd.partition_broadcast(bcs[:, :], rec[0:1, :])
                    nc.vector.tensor_mul(
                        ovT[hr:hr + DH, q0:q0 + QC], opsC[hr:hr + DH, :], bcs[:, :]
                    )

            def emit_attn_chunk(b, qc, pending, fillers=()):
                """Scores+exp for (b, qc), zippered with PV of `pending` and
                with QKV filler slices. Returns the new pending state."""
                q0 = b * N + qc * QC
                fillers = list(fillers)
                expT = spool.tile([128, NKT * 2 * QC], bf16, name="expT",
                                  tag="expT", bufs=2)
                for kt in range(NKT):
                    # both heads' score matmuls share one PSUM tile and one
                    # exp so they become ready together and stay adjacent:
                    # they are row-tiled (rows 0-63 / 64-127) and stream the
                    # same q columns, so the PE can run them concurrently.
                    psS = pspool.tile([128, 2 * QC], f32, name="psS",
                                      tag="sc", bufs=2)
                    k0 = b * N + kt * KT
                    for h in range(HPC):
                        p0, p1 = h * DH, (h + 1) * DH
                        nc.tensor.matmul(
                            psS[:, h * QC:(h + 1) * QC],
                            k_sb[p0:p1, k0:k0 + KT],
                            q_roped[p0:p1, q0:q0 + QC],
                            start=True, stop=True,
                        )
                    if fillers and kt % 2 == 0:
                        fillers.pop(0)()
                    nc.scalar.activation(
                        expT[:, 2 * kt * QC:(2 * kt + 2) * QC],
                        psS[:, :],
                        mybir.ActivationFunctionType.Exp,
                        scale=DH ** -0.5,
                    )
                    if pending is not None:
                        emit_pv_kt(pending, kt)
                        if kt == NKT - 1:
                            emit_normalize(pending)
                while fillers:
                    fillers.pop(0)()
                opsC = pspool.tile([128, QC], f32, name="opsC",
                                   tag="ops", bufs=2)
                psD = pspool.tile([128, QC], f32, name="psD",
                                  tag="ops", bufs=2)
                return (b, qc, opsC, psD, expT)

            # ---------------- collective / output helpers ---------------------
            # The A2A is split into 4 quarter-collectives (1024 tokens each),
            # launched as soon as each quarter of attention output is
            # normalized, so all but the last hide under remaining compute.
            TQ = T // 4                  # 1024 tokens per quarter
            TSQ = TQ // NCORES           # 128 tokens per core per quarter
            a2a_in = [dpool.tile([NCORES * 128, TSQ], bf16, name=f"a2a_in{q}")
                      for q in range(4)]
            a2a_out = [dpool.tile([NCORES * 128, TSQ], bf16, name=f"a2a_out{q}")
                       for q in range(4)]
            gTs = [wpool.tile([128, 8 * TSQ], bf16, name=f"gTq{q}")
                   for q in range(4)]

            def emit_a2a(q):
                nc.gpsimd.dma_start(
                    a2a_in[q].rearrange("(j p) t -> p j t", p=128),
                    ovT[:, q * TQ:(q + 1) * TQ]
                    .rearrange("p (j t) -> p j t", t=TSQ),
                )
                nc.gpsimd.collective_compute(
                    "AllToAll",
                    mybir.AluOpType.bypass,
                    ins=[a2a_in[q].opt()],
                    outs=[a2a_out[q].opt()],
                    replica_groups=[list(range(NCORES))],
                )

            def emit_gather(q, fence_tok=None):
                # fence_tok: an ovT token index whose normalize must precede
                # the gather. Keeps the scheduler from hoisting the gather
                # (and the out-proj matmuls behind it) to a point where a
                # wait on the collective would stall the in-order PE queue.
                if fence_tok is not None:
                    nc.vector.tensor_copy(
                        gTs[q][0:1, 0:8], ovT[0:1, fence_tok - 8:fence_tok]
                    )
                nc.gpsimd.dma_start(
                    gTs[q].rearrange("p (e t) -> p e t", t=TSQ),
                    a2a_out[q].rearrange("(e p) t -> p e t", p=128),
                )

            def emit_outproj(q):
                gT = gTs[q]
                for nn in range(HID // 512):
                    odps = pspool.tile([128, 512], f32, tag="mm", bufs=2)
                    for et in range(8):
                        nc.tensor.matmul(
                            odps[:, :],
                            gT[:, et * TSQ:(et + 1) * TSQ],
                            woutT[:, et * HID + nn * 512:
                                  et * HID + (nn + 1) * 512],
                            start=(et == 0),
                            stop=(et == 7),
                        )
                    osb = spool.tile([128, 512], f32, tag="osb", bufs=2)
                    nc.vector.tensor_copy(osb[:, :], odps[:, :])
                    nc.gpsimd.dma_start(
                        out_e[q * TSQ:(q + 1) * TSQ, nn * 512:(nn + 1) * 512],
                        osb[:, :],
                    )

            # ======================= emission schedule =======================
            # batch-0 QKV chunk 0; its x DMA goes ahead of the (larger) rope
            # tables so the first matmul starts ASAP.
            f0 = qkv_fillers(0)
            f0.pop(0)()                  # load_x(0)
            nc.gpsimd.dma_start(cos2[:, :], cos2_e[:, :])
            nc.gpsimd.dma_start(sin2[:, :], sin2_e[:, :])
            for f in f0:
                f()

            # Query-chunk (0,0) scores stream fine-grained behind QKV chunks
            # 1-3: each chunk's key tiles are scored as soon as that chunk's
            # k is roped, so the ACT exp pipeline starts ~20us earlier.
            expT0 = spool.tile([128, NKT * 2 * QC], bf16, name="expT",
                               tag="expT", bufs=2)

            def head_scores(kt):
                psS = pspool.tile([128, 2 * QC], f32, name="psS",
                                  tag="sc", bufs=2)
                k0 = kt * KT
                for h in range(HPC):
                    p0, p1 = h * DH, (h + 1) * DH
                    nc.tensor.matmul(
                        psS[:, h * QC:(h + 1) * QC],
                        k_sb[p0:p1, k0:k0 + KT],
                        q_roped[p0:p1, 0:QC],
                        start=True, stop=True,
                    )
                nc.scalar.activation(
                    expT0[:, 2 * kt * QC:(2 * kt + 2) * QC],
                    psS[:, :],
                    mybir.ActivationFunctionType.Exp,
                    scale=DH ** -0.5,
                )

            for c in range(1, 4):
                fill = qkv_fillers(c)
                for kt in range(4 * (c - 1), 4 * (c - 1) + 4):
                    fill.pop(0)()
                    head_scores(kt)
                    if fill:
                        fill.pop(0)()
                while fill:
                    fill.pop(0)()
                if c == 1:
                    nc.gpsimd.dma_start(
                        woutT.rearrange("p (kt c) -> p kt c", c=HID),
                        woutT_e.rearrange("(kt p) c -> p kt c", p=128),
                    )
            for kt in range(12, 16):
                head_scores(kt)
            opsC0 = pspool.tile([128, QC], f32, name="opsC", tag="ops", bufs=2)
            psD0 = pspool.tile([128, QC], f32, name="psD", tag="ops", bufs=2)
            pending = (0, 0, opsC0, psD0, expT0)

            # batch-0 attention chunks 1-3 interleaved with batch-1 QKV
            # (chunks 4-6; chunk 7 is hosted by batch-1's first chunk).
            for qc in range(1, N // QC):
                pending = emit_attn_chunk(0, qc, pending,
                                          fillers=qkv_fillers(3 + qc))
                if qc == 2:
                    emit_a2a(0)          # tokens 0:1024 normalized during qc 2

            # batch-1 attention; each remaining quarter's A2A launches as its
            # tokens are normalized; each out-proj quarter is fenced one
            # normalize behind so it fills PE slack in the ACT-bound region.
            for qc in range(N // QC):
                pending = emit_attn_chunk(
                    1, qc, pending,
                    fillers=qkv_fillers(7) if qc == 0 else ())
                if qc == 0:
                    emit_a2a(1)          # tokens 1024:2048 (norm'd during qc 0)
                if qc == 1:
                    emit_gather(0, fence_tok=N + QC)
                    emit_outproj(0)
                if qc == 2:
                    emit_a2a(2)          # tokens 2048:3072 (norm'd during qc 2)
                    emit_gather(1, fence_tok=N + 2 * QC)
                    emit_outproj(1)
            for kt in range(NKT):
                emit_pv_kt(pending, kt)
            emit_normalize(pending)
            emit_a2a(3)                  # tokens 3072:4096
            emit_gather(2, fence_tok=N + 3 * QC)
            emit_outproj(2)              # runs while A2A#3 is in flight
            emit_gather(3, fence_tok=T)
            emit_outproj(3)

    nc.finalize()
    return nc


def _host_inputs(x, rope, Wqkv, Wout):
    """Build the 8 per-core input maps with host-side layout prep."""
    xf = np.ascontiguousarray(x.reshape(T, HID).T).astype(_bf16)        # [1024, 4096]
    woutT = np.ascontiguousarray(Wout.T).astype(_bf16)                  # [1024, 1024]

    rf = rope.reshape(T, DH)                                            # [4096, 64]
    cosE = np.repeat(rf[:, 0::2], 2, axis=1).T                          # [64, 4096]
    sinE = np.repeat(rf[:, 1::2], 2, axis=1).T
    sgn = np.where(np.arange(DH) % 2 == 0, -1.0, 1.0)[:, None]
    sinS = (sinE * sgn)
    cos2 = np.ascontiguousarray(np.concatenate([cosE, cosE], 0)).astype(_bf16)
    sin2 = np.ascontiguousarray(np.concatenate([sinS, sinS], 0)).astype(_bf16)

    pm = np.zeros((128, 128), np.float32)
    for d in range(128):
        pm[d ^ 1, d] = 1.0       # partner[d] = q[d^1]; lhsT = S (symmetric)
    perm = pm.astype(_bf16)

    w3 = Wqkv.reshape(3, H, DH, HID)
    in_maps = []
    for c in range(NCORES):
        blocks = []
        for which in range(3):
            for hl in range(HPC):
                blocks.append(w3[which, 2 * c + hl])                    # [64, 1024]
        wq = np.concatenate(blocks, 0)                                  # [384, 1024]
        wqkvT = np.ascontiguousarray(wq.T).astype(_bf16)                # [1024, 384]
        in_maps.append({
            "xT": xf, "wqkvT": wqkvT, "woutT": woutT,
            "cos2": cos2, "sin2": sin2, "perm": perm,
        })
    return in_maps


_CACHE = {}


def kernel(x, rope, Wqkv, Wout):
    from concourse.bass_utils import run_bass_kernel_spmd

    if "nc" not in _CACHE:
        _CACHE["nc"] = _build_graph()
    nc = _CACHE["nc"]
    in_maps = _host_inputs(np.asarray(x, np.float32), np.asarray(rope, np.float32),
                           np.asarray(Wqkv, np.float32), np.asarray(Wout, np.float32))
    res = run_bass_kernel_spmd(nc, in_maps, core_ids=list(range(NCORES)))
    # core j returns [512, 1024]: rows q*128:(q+1)*128 hold global tokens
    # [q*1024 + j*128, q*1024 + (j+1)*128) for quarter q.
    full = np.empty((T, HID), np.float32)
    TQ, TSQ = T // 4, T // 4 // NCORES
    for j in range(NCORES):
        part = np.asarray(res.results[j]["out"], np.float32)
        for q in range(4):
            full[q * TQ + j * TSQ:q * TQ + (j + 1) * TSQ] = \
                part[q * TSQ:(q + 1) * TSQ]
    return full.reshape(B, N, HID)
